# Initial kernel scaffold
#
"""Trainium2 Bass kernel: AttentionBlock (GroupNorm + 1x1-conv QKV + MHA + proj + residual).

Data-parallel over batch: 16 samples -> 8 NeuronCores x 2 samples. Each core
runs the whole block locally (attention is per-sample, no collectives); the
host shards inputs and concatenates the 8 output shards.

Math notes (exact rewrites, not approximations):
  - scores are computed transposed, S^T[m,n] = sum_d k[d,m] q'[d,n] with
    q' = (q + b_q) * d^-0.5. The k-bias adds a column-constant to S^T which
    softmax cancels, so it is dropped.
  - softmax denominator comes from a ones-column appended to v^T in the
    attn@v matmul (row 64 of the [65, n] output accumulates colsum(exp S^T)).
  - v-bias: attn rows sum to 1, so  attn @ (Wv h + bv) = attn @ Wv h + bv;
    the bv term is folded into the proj bias on the host:
    beff = b_proj + w_proj @ bv.
"""

import os
import sys
import types
from contextlib import ExitStack

import ml_dtypes
import numpy as np

# If BASS_TRACE is set but this container's antenv lacks the NTFF hook
# module, bass_utils' trace path would crash on import; give it a null
# hook so tracing degrades gracefully instead.
try:
    import antenv.axon_hooks  # noqa: F401
except Exception:  # pragma: no cover
    try:
        import antenv

        _hookmod = types.ModuleType("antenv.axon_hooks")
        _hook = [None]
        _hookmod.set_axon_ntff_profile_hook = lambda h: _hook.__setitem__(0, h)
        _hookmod.get_axon_ntff_profile_hook = lambda: _hook[0]
        sys.modules["antenv.axon_hooks"] = _hookmod
        antenv.axon_hooks = _hookmod
    except Exception:
        pass

import concourse.bass as bass
import concourse.tile as tile
from concourse import bacc
from concourse import mybir
from concourse.bass_utils import run_bass_kernel_spmd

F32 = mybir.dt.float32
BF16 = mybir.dt.bfloat16
AF = mybir.ActivationFunctionType
ALU = mybir.AluOpType

# Problem dims (hardcoded per spec: x [16, 512, 32, 32] f32)
B, C, H, W = 16, 512, 32, 32
N = H * W                # 1024 spatial positions
NCORES = 8
BS = B // NCORES         # 2 samples per core
G = 32                   # groupnorm groups
HEADS = 8
D = C // HEADS           # 64
CT = C // 128            # 4 channel tiles
MT = N // 128            # 8 m-tiles (spatial, attention contraction)
NHALF = 2                # n split in halves of 512 (psum bank limit)
EPS = 1e-5
GROUP_ELEMS = (C // G) * N   # 16 ch * 1024 = 16384 per group

LAST_EXEC_NS = None
LAST_RESULTS = None


def _build_tile(ctx: ExitStack, tc: tile.TileContext, te: dict):
    nc = tc.nc
    x_e, out_e = te["x"], te["out"]

    const = ctx.enter_context(tc.tile_pool(name="const", bufs=1))
    small = ctx.enter_context(tc.tile_pool(name="small", bufs=6))
    ps_acc = ctx.enter_context(tc.tile_pool(name="ps_acc", bufs=4, space="PSUM"))
    ps_sc = ctx.enter_context(tc.tile_pool(name="ps_sc", bufs=2, space="PSUM"))

    # ---- groupnorm stats over the [BS*G, 16384] view of x ----
    # Issued before the weight loads: the stats chain gates the first matmul.
    # Chunked DMAs so bn_stats tracks the stream instead of one 4MB barrier.
    NCHUNK = GROUP_ELEMS // 512          # bn_stats hw max free = 512
    GCH = 4
    stats_sb = const.tile([BS * G, NCHUNK, 6], F32)
    eps_sb = const.tile([BS * G, 1], F32)
    nc.vector.memset(eps_sb[:], EPS)
    # preload the Exp ACT table set off the critical path
    dummy_act = const.tile([1, 1], F32)
    nc.scalar.activation(dummy_act[:], eps_sb[0:1, :], AF.Exp)
    # stats on [128, 8192] half-group rows: full-width DMA ports (a [64, N]
    # layout would halve DMA bandwidth) and half the bn_stats calls
    HSUB = GROUP_ELEMS // 2 // 512 // GCH  # 512-wide bn_stats per DMA chunk
    stats2 = const.tile([128, GROUP_ELEMS // 2 // 512, 6], F32)
    with tc.tile_pool(name="gnx", bufs=2) as gnxp:
        for gc in range(GCH):
            gnx = gnxp.tile([128, HSUB, 512], F32, tag="gnx", name="gnx")
            in_ap = bass.AP(
                tensor=x_e,
                offset=gc * HSUB * 512,
                ap=[[C * N, BS], [GROUP_ELEMS // 2, 2 * G], [1, HSUB * 512]],
            )
            nc.sync.dma_start(out=gnx[:], in_=in_ap)
            for j in range(HSUB):
                nc.vector.bn_stats(out=stats2[:, gc * HSUB + j, :], in_=gnx[:, j, :])
    # fold half-group stats rows back to [group, 2*chunks] (both sides are
    # contiguous, single sbuf-to-sbuf DMA), then aggregate per group
    nc.gpsimd.dma_start(out=stats_sb[:], in_=stats2[:])

    # ---- constants / weights to SBUF (needed ~30us in; loads overlap stats) ----
    wqk_sb = const.tile([128, CT, 2 * C], BF16)   # w_qkv[:1024].T tiles
    wv_sb = const.tile([128, CT, C], BF16)        # w_qkv[1024:].T tiles
    wp_sb = const.tile([128, CT, C], BF16)        # w_proj.T tiles
    bq_sb = const.tile([128, CT, 1], F32)
    beff_sb = const.tile([128, CT, 1], F32)
    for kt in range(CT):
        sl = slice(kt * 128, (kt + 1) * 128)
        nc.sync.dma_start(out=wqk_sb[:, kt, :], in_=te["wqkT"][sl, :])
        nc.sync.dma_start(out=wv_sb[:, kt, :], in_=te["wvT"][sl, :])
        nc.sync.dma_start(out=wp_sb[:, kt, :], in_=te["wpT"][sl, :])
        nc.sync.dma_start(out=bq_sb[:, kt, :], in_=te["bq"][sl, :])
        nc.sync.dma_start(out=beff_sb[:, kt, :], in_=te["beff"][sl, :])
    # gamma/beta replicated per sample: [128, (s, t)] layout
    gam2 = const.tile([128, BS * CT], F32)
    bet2 = const.tile([128, BS * CT], F32)
    for s in range(BS):
        nc.sync.dma_start(
            out=gam2[:, s * CT : (s + 1) * CT],
            in_=bass.AP(tensor=te["gamma"], offset=0, ap=[[1, 128], [128, CT]]),
        )
        nc.sync.dma_start(
            out=bet2[:, s * CT : (s + 1) * CT],
            in_=bass.AP(tensor=te["beta"], offset=0, ap=[[1, 128], [128, CT]]),
        )

    mv = small.tile([BS * G, 2], F32, tag="mv")
    nc.vector.bn_aggr(out=mv[:], in_=stats_sb[:])
    # rstd = rsqrt(var + eps) via int-seed + 2 Newton steps, all on DVE —
    # keeps ACT on the single Exp table set for the whole kernel (Ln/Sqrt
    # would force table reloads).
    I32 = mybir.dt.int32
    st2 = small.tile([BS * G, 2], mybir.dt.float32r, tag="st2")
    nc.vector.tensor_copy(st2[:, 0:1], mv[:, 0:1])
    vpe = small.tile([BS * G, 1], F32, tag="vpe")
    nc.vector.tensor_scalar_add(vpe[:], mv[:, 1:2], EPS)
    hv = small.tile([BS * G, 1], F32, tag="hv")
    nc.vector.tensor_scalar_mul(hv[:], vpe[:], -0.5)
    y0 = small.tile([BS * G, 1], F32, tag="y0")
    ysh = small.tile([BS * G, 1], I32, tag="ysh")
    nc.vector.tensor_scalar(
        out=ysh[:],
        in0=vpe[:].bitcast(I32),
        scalar1=1,
        scalar2=None,
        op0=ALU.arith_shift_right,
    )
    nc.vector.tensor_scalar(
        out=y0[:].bitcast(I32),
        in0=ysh[:],
        scalar1=-1,
        scalar2=0x5F3759DF,
        op0=ALU.mult,
        op1=ALU.add,
    )
    y1 = small.tile([BS * G, 1], F32, tag="y1")
    yw = small.tile([BS * G, 1], F32, tag="yw")
    # Newton: y <- y * (1.5 - 0.5*v*y^2), twice
    nc.vector.tensor_mul(yw[:], y0[:], y0[:])
    nc.vector.tensor_mul(yw[:], yw[:], hv[:])
    nc.vector.tensor_scalar_add(yw[:], yw[:], 1.5)
    nc.vector.tensor_mul(y1[:], y0[:], yw[:])
    nc.vector.tensor_mul(yw[:], y1[:], y1[:])
    nc.vector.tensor_mul(yw[:], yw[:], hv[:])
    nc.vector.tensor_scalar_add(yw[:], yw[:], 1.5)
    nc.vector.tensor_mul(st2[:, 1:2], y1[:], yw[:])

    # broadcast group stats to channel vectors with tiny selector matmuls on
    # the (otherwise idle) PE: mvr[p, j, :] = (mean, rstd) of group g(p, j);
    # SEL comes from the host, f32r keeps the stats at ~f32 precision
    F32R = mybir.dt.float32r
    A_all = const.tile([128, BS * CT], F32)
    B_all = const.tile([128, BS * CT], F32)
    with tc.tile_pool(name="selp", bufs=1) as selp:
        sel_sb = selp.tile([BS * G, BS * CT, 128], F32R)
        nc.sync.dma_start(
            out=sel_sb[:], in_=te["sel"][:].rearrange("g (j p) -> g j p", p=128)
        )
        mvr_ps = ps_sc.tile([128, BS * CT, 2], F32, tag="sc", name="mvr_ps")
        for j in range(BS * CT):
            nc.tensor.matmul(
                mvr_ps[:, j, :],
                sel_sb[:, j, :],
                st2[:],
                start=True,
                stop=True,
            )
        # h = x*A + Bv over all (s, t): A = rstd*gamma, Bv = beta - mean*A
        nc.vector.tensor_mul(A_all[:], mvr_ps[:, :, 1], gam2[:])
        tmpA = small.tile([128, BS * CT], F32, tag="tmpA")
        nc.vector.tensor_mul(tmpA[:], mvr_ps[:, :, 0], A_all[:])
        nc.vector.tensor_sub(B_all[:], bet2[:], tmpA[:])

    # Main pools open after the gn-stats/sel pools have freed their space.
    xpool = ctx.enter_context(tc.tile_pool(name="xres", bufs=1))
    hpool = ctx.enter_context(tc.tile_pool(name="h", bufs=1))
    qkpool = ctx.enter_context(tc.tile_pool(name="qk", bufs=2))
    vtpool = ctx.enter_context(tc.tile_pool(name="vt", bufs=2))
    atpool = ctx.enter_context(tc.tile_pool(name="attn", bufs=2))
    aopool = ctx.enter_context(tc.tile_pool(name="ao", bufs=2))
    rbpool = ctx.enter_context(tc.tile_pool(name="rb", bufs=4))
    rcppool = ctx.enter_context(tc.tile_pool(name="rcps", bufs=2))
    outpool = ctx.enter_context(tc.tile_pool(name="outp", bufs=2))

    x_sb = xpool.tile([128, BS * CT, N], F32)
    for s in range(BS):
        for t in range(CT):
            nc.gpsimd.dma_start(
                out=x_sb[:, s * CT + t, :], in_=x_e[s, t * 128 : (t + 1) * 128, :]
            )

    def emit_prep(s):
        # ---- groupnorm apply -> h (bf16), then qk / vT matmuls ----
        h_sb = hpool.tile([128, CT, N], BF16, tag="h", name="h_sb")
        for t in range(CT):
            if t % 2 == 0:  # split h-apply across ACT and DVE
                nc.scalar.activation(
                    h_sb[:, t, :],
                    x_sb[:, s * CT + t, :],
                    AF.Identity,
                    bias=B_all[:, s * CT + t : s * CT + t + 1],
                    scale=A_all[:, s * CT + t : s * CT + t + 1],
                )
            else:
                nc.vector.tensor_scalar(
                    out=h_sb[:, t, :],
                    in0=x_sb[:, s * CT + t, :],
                    scalar1=A_all[:, s * CT + t : s * CT + t + 1],
                    scalar2=B_all[:, s * CT + t : s * CT + t + 1],
                    op0=ALU.mult,
                    op1=ALU.add,
                )

        # ---- qk = wqkT.T @ h   ([o, n], o-tile p holds heads 2p, 2p+1) ----
        q_sb = qkpool.tile([128, CT, N], BF16, tag="q", name="q_sb")
        k_sb = qkpool.tile([128, CT, N], BF16, tag="k", name="k_sb")
        for o in range(2 * CT):
            for nh in range(NHALF):
                ps = ps_acc.tile([128, 512], F32, tag="acc", name="ps")
                for kt in range(CT):
                    nc.tensor.matmul(
                        ps[:],
                        wqk_sb[:, kt, o * 128 : (o + 1) * 128],
                        h_sb[:, kt, nh * 512 : (nh + 1) * 512],
                        start=(kt == 0),
                        stop=(kt == CT - 1),
                    )
                if o < CT:  # q channels: scale+bias fused into the copy
                    nc.vector.tensor_scalar(
                        out=q_sb[:, o, nh * 512 : (nh + 1) * 512],
                        in0=ps[:],
                        scalar1=bq_sb[:, o, :],
                        scalar2=float(D) ** -0.5,
                        op0=ALU.add,
                        op1=ALU.mult,
                    )
                else:  # k channels: plain copy (bias dropped, see header)
                    nc.vector.tensor_copy(
                        k_sb[:, o - CT, nh * 512 : (nh + 1) * 512], ps[:]
                    )

        # ---- vT = h.T @ wvT  ([m, dv] + ones column for colsum) ----
        vt_sb = vtpool.tile([128, MT, HEADS, D + 1], BF16, tag="vt")
        nc.vector.memset(vt_sb[:, :, :, D : D + 1], 1.0)
        for m in range(MT):
            ps = ps_acc.tile([128, 512], F32, tag="acc", name="ps")
            for kt in range(CT):
                nc.tensor.matmul(
                    ps[:],
                    h_sb[:, kt, m * 128 : (m + 1) * 128],
                    wv_sb[:, kt, :],
                    start=(kt == 0),
                    stop=(kt == CT - 1),
                )
            nc.vector.tensor_copy(
                vt_sb[:, m, :, 0:D], ps[:].rearrange("p (h d) -> p h d", h=HEADS)
            )

        return q_sb, k_sb, vt_sb

    def emit_attention(s, q_sb, k_sb, vt_sb):
        # ---- attention: QK/exp of pair p interleaved with AV of pair p-1 ----
        # (fills the PE gaps while ACT runs exp; ~2x denser PE stream)
        ao_sb = aopool.tile([128, CT, N], BF16, tag="ao", name="ao_sb")

        def emit_av_chunk(prev_state, m):
            p0, at0, avs0 = prev_state
            for hh in range(2):
                for nh in range(NHALF):
                    nc.tensor.matmul(
                        avs0[hh][nh][:],
                        vt_sb[:, m, 2 * p0 + hh, :],
                        at0[:, hh, m, nh * 512 : (nh + 1) * 512],
                        start=(m == 0),
                        stop=(m == MT - 1),
                    )

        def emit_normalize(prev_state):
            p0, at0, avs0 = prev_state
            for hh in range(2):
                for nh in range(NHALF):
                    nsl = slice(nh * 512, (nh + 1) * 512)
                    # custom-DVE recip misreads PSUM sources on HW: SBUF-bounce
                    cs = rcppool.tile([1, 512], F32, tag="cs", name="cs")
                    nc.vector.tensor_copy(cs[:], avs0[hh][nh][D : D + 1, :])
                    rcp = rcppool.tile([1, 512], F32, tag="rcp", name="rcp")
                    nc.vector.reciprocal_approx_fast(rcp[:], cs[:])
                    rb = rbpool.tile([64, 512], F32, tag="rb", name="rb")
                    nc.gpsimd.partition_broadcast(rb[:], rcp[:])
                    nc.vector.tensor_mul(
                        ao_sb[hh * 64 : (hh + 1) * 64, p0, nsl],
                        avs0[hh][nh][0:D, :],
                        rb[:],
                    )

        prev = None
        for p in range(HEADS // 2):
            at_pair = atpool.tile([128, 2, MT, N], BF16, tag="attn", name="at_pair")
            for m in range(MT):
                for hh in range(2):
                    base = hh * 64
                    sc = ps_sc.tile([128, N], F32, tag="sc", name="sc")
                    for nh in range(NHALF):
                        nsl = slice(nh * 512, (nh + 1) * 512)
                        nc.tensor.matmul(
                            sc[:, nsl],
                            k_sb[base : base + 64, p, m * 128 : (m + 1) * 128],
                            q_sb[base : base + 64, p, nsl],
                            start=True,
                            stop=True,
                            tile_position=(base, 0),
                        )
                    nc.scalar.activation(at_pair[:, hh, m, :], sc[:], AF.Exp)
                if prev is not None:
                    emit_av_chunk(prev, m)
            if prev is not None:
                emit_normalize(prev)
            if p < HEADS // 2 - 1:
                avs = [
                    [
                        ps_acc.tile([D + 1, 512], F32, tag="acc", name=f"av{hh}_{nh}")
                        for nh in range(NHALF)
                    ]
                    for hh in range(2)
                ]
            else:
                # drain pair accumulates in the (then idle) scores pool so the
                # acc pool frees for the next sample's qkv before normalize
                dr = [
                    ps_sc.tile([D + 1, N], F32, tag="sc", name=f"drain{hh}")
                    for hh in range(2)
                ]
                avs = [
                    [dr[hh][:, nh * 512 : (nh + 1) * 512] for nh in range(NHALF)]
                    for hh in range(2)
                ]
            prev = (p, at_pair, avs)
        for m in range(MT):
            emit_av_chunk(prev, m)
        emit_normalize(prev)
        return ao_sb

    def emit_proj(s, ao_sb):
        # ---- proj + bias + residual, two waves of 4 open psum groups ----
        # kt=0..2 partials need only pairs 0-2's ao, so they run while the
        # drain pair's normalize chain resolves; kt=3 closes each group.
        for wave in range(2):
            pss = []
            for t in range(wave * 2, wave * 2 + 2):
                for nh in range(NHALF):
                    nsl = slice(nh * 512, (nh + 1) * 512)
                    ps = ps_acc.tile(
                        [128, 512], F32, tag="acc", name=f"pj{t}_{nh}"
                    )
                    pss.append((t, nh, nsl, ps))
                    for kt in range(CT - 1):
                        nc.tensor.matmul(
                            ps[:],
                            wp_sb[:, kt, t * 128 : (t + 1) * 128],
                            ao_sb[:, kt, nsl],
                            start=(kt == 0),
                            stop=False,
                        )
            for t, nh, nsl, ps in pss:
                nc.tensor.matmul(
                    ps[:],
                    wp_sb[:, CT - 1, t * 128 : (t + 1) * 128],
                    ao_sb[:, CT - 1, nsl],
                    start=False,
                    stop=True,
                )
                ot = outpool.tile([128, 512], F32, tag="out", name="ot")
                nc.vector.scalar_tensor_tensor(
                    out=ot[:],
                    in0=ps[:],
                    scalar=beff_sb[:, t, :],
                    in1=x_sb[:, s * CT + t, nsl],
                    op0=ALU.add,
                    op1=ALU.add,
                )
                nc.gpsimd.dma_start(
                    out=out_e[s, t * 128 : (t + 1) * 128, nsl], in_=ot[:]
                )

    # Drive: emit next sample's qkv prep between a sample's attention drain
    # and its proj, so the PE instruction stream has work while the
    # normalize (recip -> DRAM bounce -> broadcast) latency resolves.
    tiles = emit_prep(0)
    for s in range(BS):
        ao = emit_attention(s, *tiles)
        if s + 1 < BS:
            tiles = emit_prep(s + 1)
        emit_proj(s, ao)


def build_bass() -> bass.Bass:
    nc = bacc.Bacc()
    te = {
        "x": nc.declare_dram_parameter("x", [BS, C, N], F32, isOutput=False),
        "wqkT": nc.declare_dram_parameter("wqkT", [C, 2 * C], BF16, isOutput=False),
        "wvT": nc.declare_dram_parameter("wvT", [C, C], BF16, isOutput=False),
        "wpT": nc.declare_dram_parameter("wpT", [C, C], BF16, isOutput=False),
        "bq": nc.declare_dram_parameter("bq", [C, 1], F32, isOutput=False),
        "beff": nc.declare_dram_parameter("beff", [C, 1], F32, isOutput=False),
        "gamma": nc.declare_dram_parameter("gamma", [C, 1], F32, isOutput=False),
        "beta": nc.declare_dram_parameter("beta", [C, 1], F32, isOutput=False),
        "sel": nc.declare_dram_parameter(
            "sel", [BS * G, BS * CT * 128], mybir.dt.float32r, isOutput=False
        ),
        "out": nc.declare_dram_parameter("out", [BS, C, N], F32, isOutput=True),
    }
    with tile.TileContext(nc) as tc:
        with ExitStack() as ctx:
            _build_tile(ctx, tc, te)
    # Bacc defers register allocation to finalize(); run_bass_via_pjrt
    # serializes the module without calling it, so do it here.
    nc.finalize()
    return nc


def _make_sel() -> np.ndarray:
    sel = np.zeros((BS * G, BS * CT, 128), np.float32)
    for j in range(BS * CT):
        s0, t0 = j // CT, j % CT
        for p in range(128):
            sel[s0 * G + t0 * 8 + p // 16, j, p] = 1.0
    return sel.reshape(BS * G, BS * CT * 128)


def make_in_maps(inputs: dict) -> list[dict]:
    x = np.ascontiguousarray(np.asarray(inputs["x"], np.float32)).reshape(B, C, N)
    w_qkv = np.asarray(inputs["w_qkv"], np.float32)
    b_qkv = np.asarray(inputs["b_qkv"], np.float32)
    w_proj = np.asarray(inputs["w_proj"], np.float32)
    b_proj = np.asarray(inputs["b_proj"], np.float32)
    gamma = np.asarray(inputs["gamma"], np.float32)
    beta = np.asarray(inputs["beta"], np.float32)

    bf = ml_dtypes.bfloat16
    common = {
        "wqkT": np.ascontiguousarray(w_qkv[: 2 * C, :].T).astype(bf),
        "wvT": np.ascontiguousarray(w_qkv[2 * C :, :].T).astype(bf),
        "wpT": np.ascontiguousarray(w_proj.T).astype(bf),
        "bq": b_qkv[:C].reshape(C, 1).copy(),
        "beff": (b_proj + w_proj @ b_qkv[2 * C :]).reshape(C, 1).astype(np.float32),
        "gamma": gamma.reshape(C, 1).copy(),
        "beta": beta.reshape(C, 1).copy(),
        "sel": _make_sel(),
    }
    return [
        {"x": np.ascontiguousarray(x[i * BS : (i + 1) * BS]), **common}
        for i in range(NCORES)
    ]


def kernel(**inputs) -> np.ndarray:
    global LAST_EXEC_NS, LAST_RESULTS
    nc = build_bass()
    in_maps = make_in_maps(inputs)
    res = run_bass_kernel_spmd(nc, in_maps, list(range(NCORES)))
    LAST_RESULTS = res
    LAST_EXEC_NS = res.exec_time_ns
    out = np.concatenate([np.asarray(res.results[i]["out"]) for i in range(NCORES)], 0)
    return out.reshape(B, C, H, W).astype(np.float32)



# revision 2
# speedup vs baseline: 1.2056x; 1.2056x over previous
"""Trainium2 Bass kernel v2: AttentionBlock (GroupNorm + 1x1-conv QKV + MHA + proj + residual).

Data-parallel over batch: 16 samples -> 8 NeuronCores x 2 samples. Each core
runs the whole block locally; host shards inputs / concats outputs.

v2 changes vs v1 (335us):
  - fp8e4 DoubleRow matmuls for qkv / v / attn@v / proj: one instruction
    contracts 256 (2 k-tiles), halving PE streaming time for those stages.
    Scores stay bf16 (they are PSUM-output-bound; fp8 wouldn't help).
  - h, v^T, attention weights (exp scores), and attention output are fp8e4;
    q/k stay bf16 for the scores matmul.
  - GroupNorm stats come from the resident x tiles (x is loaded once, not
    twice): per-partition bn_stats/bn_aggr, then a [128,128] block-averaging
    f32r matmul broadcasts group (mean, E[x^2]) back to every partition.
  - Scores matmul emits the full [128, 1024] row block in one instruction
    (2 PSUM banks), halving scores instruction count; exp runs on
    [128, 1024] ACTIVATEs.
  - Software pipeline: sample s+1's qkv prep and sample s-1's proj are
    interleaved into the attention pair loop at m-step granularity to keep
    the PE stream dense (HAM stays un-throttled).

Math notes (exact rewrites, not approximations):
  - scores computed transposed, S^T[m,n] = sum_d k[d,m] q'[d,n], with
    q' = (q + b_q) * d^-0.5; k-bias drops (softmax-invariant).
  - softmax denominator comes from a ones-column appended to v^T (row 64 of
    the [65, n] attn@v output accumulates colsum(exp S^T)).
  - v-bias folded into proj bias on host: beff = b_proj + w_proj @ bv.
"""

import os
import sys
import types
from contextlib import ExitStack

import ml_dtypes
import numpy as np

# If BASS_TRACE is set but this container's antenv lacks the NTFF hook
# module, bass_utils' trace path would crash on import; give it a null
# hook so tracing degrades gracefully instead.
try:
    import antenv.axon_hooks  # noqa: F401
except Exception:  # pragma: no cover
    try:
        import antenv

        _hookmod = types.ModuleType("antenv.axon_hooks")
        _hook = [None]
        _hookmod.set_axon_ntff_profile_hook = lambda h: _hook.__setitem__(0, h)
        _hookmod.get_axon_ntff_profile_hook = lambda: _hook[0]
        sys.modules["antenv.axon_hooks"] = _hookmod
        antenv.axon_hooks = _hookmod
    except Exception:
        pass

import concourse.bass as bass
import concourse.tile as tile
from concourse import bacc
from concourse import mybir
from concourse.bass_utils import run_bass_kernel_spmd

F32 = mybir.dt.float32
F32R = mybir.dt.float32r
BF16 = mybir.dt.bfloat16
FP8 = mybir.dt.float8e4
FP8E5 = mybir.dt.float8e5  # at: e5m2's range makes exp overflow-safe, no bias needed
I32 = mybir.dt.int32
AF = mybir.ActivationFunctionType
ALU = mybir.AluOpType
DR = mybir.MatmulPerfMode.DoubleRow

# Problem dims (hardcoded per spec: x [16, 512, 32, 32] f32)
B, C, H, W = 16, 512, 32, 32
N = H * W                # 1024 spatial positions
NCORES = 8
BS = B // NCORES         # 2 samples per core
G = 32                   # groupnorm groups (16 channels each)
HEADS = 8
D = C // HEADS           # 64
CT = C // 128            # 4 channel tiles
MT = N // 128            # 8 m-tiles (attention contraction)
EPS = 1e-5
VW = 72                  # v^T row pitch: 64 d + ones col + pad (16B-aligned)
SCALE = float(D) ** -0.5

LAST_EXEC_NS = None
LAST_RESULTS = None


def _build_tile(ctx: ExitStack, tc: tile.TileContext, te: dict):
    nc = tc.nc
    x_e, out_e = te["x"], te["out"]

    const = ctx.enter_context(tc.tile_pool(name="const", bufs=1))
    small = ctx.enter_context(tc.tile_pool(name="small", bufs=2))
    xpool = ctx.enter_context(tc.tile_pool(name="xres", bufs=1))
    hpool = ctx.enter_context(tc.tile_pool(name="h", bufs=2))
    qkpool = ctx.enter_context(tc.tile_pool(name="qk", bufs=2))
    vtpool = ctx.enter_context(tc.tile_pool(name="vt", bufs=2))
    atpool = ctx.enter_context(tc.tile_pool(name="attn", bufs=3))
    aopool = ctx.enter_context(tc.tile_pool(name="ao", bufs=2))
    rcppool = ctx.enter_context(tc.tile_pool(name="rcps", bufs=4))
    rbpool = ctx.enter_context(tc.tile_pool(name="rb", bufs=4))
    outwpool = ctx.enter_context(tc.tile_pool(name="outw", bufs=3))
    # PSUM: one unified pool of 3x[128,1024] (6 banks) feeds scores AND the
    # qkv/v/proj accumulators (each unit claims a tile and uses both bank
    # halves), plus 2x[65,512] attn@v accumulators = 8 banks.
    ps_sc = ctx.enter_context(tc.tile_pool(name="ps_sc", bufs=3, space="PSUM"))
    ps_av = ctx.enter_context(tc.tile_pool(name="ps_av", bufs=2, space="PSUM"))

    # ---- constants / weights to SBUF ----
    eps_sb = const.tile([1, 1], F32)
    nc.vector.memset(eps_sb[:], EPS)
    dummy_act = const.tile([1, 1], F32)
    nc.scalar.activation(dummy_act[:], eps_sb[:], AF.Exp)  # preload Exp tables

    # Everything streams over the sync HW-DGE queue in dependency order:
    # sample-0 x tiles (contiguous 512KB each; the groupnorm-stats critical
    # path), then the weights, then sample-1 x.
    wqk_sb = const.tile([128, CT, 2 * C], FP8)   # w_qkv[:1024].T k-tiles
    wv_sb = const.tile([128, CT, C], FP8)        # w_qkv[1024:].T k-tiles
    wp_sb = const.tile([128, CT, C], FP8)        # w_proj.T k-tiles
    bq_sb = const.tile([128, CT, 1], F32)
    beff_sb = const.tile([128, CT, 1], F32)
    gam2 = const.tile([128, CT], F32)
    bet2 = const.tile([128, CT], F32)
    selT_sb = const.tile([128, 128], F32R)       # (1/16) * blockdiag(ones 16x16)
    A_all = const.tile([128, BS * CT], F32)
    B_all = const.tile([128, BS * CT], F32)
    stats2 = const.tile([128, BS * CT, 2, 6], F32)

    x_t = [[None] * CT for _ in range(BS)]

    def emit_x_dmas(s):
        for t in range(CT):
            xt = xpool.tile([128, N], F32, tag=f"x{s}{t}", name=f"x{s}{t}")
            nc.sync.dma_start(out=xt[:], in_=x_e[s, t * 128 : (t + 1) * 128, :])
            x_t[s][t] = xt

    # weights go over the scalar HW-DGE queue (ACT is idle in the lead-in)
    # in parallel with x streaming on the sync queue; host pre-tiles the
    # weights to [128, kt, o] so each load is one fully contiguous DMA.
    nc.scalar.dma_start(out=gam2[:], in_=te["gamma"][:])
    nc.scalar.dma_start(out=bet2[:], in_=te["beta"][:])
    nc.scalar.dma_start(out=selT_sb[:], in_=te["selT"][:])
    nc.scalar.dma_start(out=wqk_sb[:], in_=te["wqkT"][:])
    nc.scalar.dma_start(out=bq_sb[:], in_=te["bq"][:])
    nc.scalar.dma_start(out=wv_sb[:], in_=te["wvT"][:])
    nc.scalar.dma_start(out=wp_sb[:], in_=te["wpT"][:])
    nc.scalar.dma_start(out=beff_sb[:], in_=te["beff"][:])
    emit_x_dmas(0)
    emit_x_dmas(1)

    mv_t = [None, None]

    def emit_stats(s):
        # bn_stats per x tile chunk (hw free max 512); bn_aggr interleaved
        # per tile so the aggregation overlaps the next tile's DMA
        mv = small.tile([128, CT, 2], F32, tag="mv")
        mv_t[s] = mv
        for t in range(CT):
            for c in range(2):
                nc.vector.bn_stats(
                    out=stats2[:, s * CT + t, c, :],
                    in_=x_t[s][t][:, c * 512 : (c + 1) * 512],
                )
            nc.vector.bn_aggr(out=mv[:, t, :], in_=stats2[:, s * CT + t, :, :])

    def emit_ab(s):
        # per-partition (mean, var) per channel tile -> group stats via the
        # block-averaging matmul -> A = gamma*rstd, B = beta - mean*A
        mv = mv_t[s]
        rhs = small.tile([128, 2 * CT], F32R, tag="rhs")
        m2 = small.tile([128, CT], F32, tag="m2")
        nc.vector.tensor_copy(rhs[:, 0:CT], mv[:, :, 0])
        nc.vector.tensor_mul(m2[:], mv[:, :, 0], mv[:, :, 0])
        nc.vector.tensor_tensor(
            out=rhs[:, CT : 2 * CT], in0=mv[:, :, 1], in1=m2[:], op=ALU.add
        )
        gps = ps_sc.tile([128, N], F32, tag="sc", name=f"gst{s}")
        nc.tensor.matmul(gps[:, 0 : 2 * CT], selT_sb[:], rhs[:], start=True, stop=True)
        gst = small.tile([128, 2 * CT], F32, tag="gst")
        nc.vector.tensor_copy(gst[:], gps[:, 0 : 2 * CT])
        # vpe = Ex2_g - mean_g^2 + eps ; rstd = rsqrt(vpe) via int-seed Newton
        vpe = small.tile([128, CT], F32, tag="vpe")
        nc.vector.tensor_mul(m2[:], gst[:, 0:CT], gst[:, 0:CT])
        nc.vector.tensor_sub(vpe[:], gst[:, CT : 2 * CT], m2[:])
        nc.vector.tensor_scalar_add(vpe[:], vpe[:], EPS)
        hv = small.tile([128, CT], F32, tag="hv")
        nc.vector.tensor_scalar_mul(hv[:], vpe[:], -0.5)
        ysh = small.tile([128, CT], I32, tag="ysh")
        y0 = small.tile([128, CT], F32, tag="y0")
        nc.vector.tensor_scalar(
            out=ysh[:],
            in0=vpe[:].bitcast(I32),
            scalar1=1,
            scalar2=None,
            op0=ALU.arith_shift_right,
        )
        nc.vector.tensor_scalar(
            out=y0[:].bitcast(I32),
            in0=ysh[:],
            scalar1=-1,
            scalar2=0x5F3759DF,
            op0=ALU.mult,
            op1=ALU.add,
        )
        # one Newton step: seed ~3.4% -> ~0.17% rel err, noise floor next to
        # the fp8 h quantization (3.6%)
        rstd = small.tile([128, CT], F32, tag="rstd")
        yw = small.tile([128, CT], F32, tag="yw")
        nc.vector.tensor_mul(yw[:], y0[:], y0[:])
        nc.vector.tensor_mul(yw[:], yw[:], hv[:])
        nc.vector.tensor_scalar_add(yw[:], yw[:], 1.5)
        nc.vector.tensor_mul(rstd[:], y0[:], yw[:])
        asl = slice(s * CT, (s + 1) * CT)
        nc.vector.tensor_mul(A_all[:, asl], rstd[:], gam2[:])
        tmpA = small.tile([128, CT], F32, tag="tmpA")
        nc.vector.tensor_mul(tmpA[:], gst[:, 0:CT], A_all[:, asl])
        nc.vector.tensor_sub(B_all[:, asl], bet2[:], tmpA[:])

    def emit_qk_unit(tiles_s, o):
        h_sb, q_sb, k_sb, _ = tiles_s
        ps = ps_sc.tile([128, N], F32, tag="sc", name="qkps")
        for nh in range(2):
            nsl = slice(nh * 512, (nh + 1) * 512)
            for kp in range(2):
                nc.tensor.matmul(
                    ps[:, nsl],
                    wqk_sb[:, 2 * kp : 2 * kp + 2, o * 128 : (o + 1) * 128],
                    h_sb[:, 2 * kp : 2 * kp + 2, nsl],
                    start=(kp == 0),
                    stop=(kp == 1),
                    perf_mode=DR,
                )
        if o < CT:  # q: bias + scale fused into the copy
            nc.vector.tensor_scalar(
                out=q_sb[:, o, :],
                in0=ps[:],
                scalar1=bq_sb[:, o, :],
                scalar2=SCALE,
                op0=ALU.add,
                op1=ALU.mult,
            )
        else:  # k: plain copy (bias dropped)
            nc.vector.tensor_copy(k_sb[:, o - CT, :], ps[:])

    def emit_v_unit(tiles_s, mp):
        h_sb, _, _, vt_sb = tiles_s
        ps = ps_sc.tile([128, N], F32, tag="sc", name="vps")
        for j in range(2):
            jsl = slice(j * 512, (j + 1) * 512)
            for kp in range(2):
                nc.tensor.matmul(
                    ps[:, jsl],
                    h_sb[:, 2 * kp : 2 * kp + 2, (mp + j) * 128 : (mp + j + 1) * 128],
                    wv_sb[:, 2 * kp : 2 * kp + 2, :],
                    start=(kp == 0),
                    stop=(kp == 1),
                    perf_mode=DR,
                )
        nc.vector.tensor_copy(
            vt_sb[:, mp : mp + 2, :, 0:D],
            ps[:].rearrange("p (m2 h d) -> p m2 h d", m2=2, h=HEADS),
        )

    def alloc_prep_tiles(s, out_tiles):
        h_sb = hpool.tile([128, CT, N], FP8, tag="h", name=f"h{s}")
        q_sb = qkpool.tile([128, CT, N], BF16, tag="q", name=f"q{s}")
        k_sb = qkpool.tile([128, CT, N], BF16, tag="k", name=f"k{s}")
        vt_sb = vtpool.tile([128, MT, HEADS, VW], FP8, tag="vt", name=f"vt{s}")
        nc.vector.memset(vt_sb[:, :, :, D : D + 1], 1.0)
        out_tiles[s] = (q_sb, k_sb, vt_sb)
        return (h_sb, q_sb, k_sb, vt_sb)

    def emit_h(s, tiles_s, use_act=False):
        # use_act splits the groupnorm apply across ACT and DVE (lead-in,
        # where ACT is otherwise idle and the h chain gates the first matmul)
        for t in range(CT):
            j = s * CT + t
            if use_act and t % 2 == 0:
                nc.scalar.activation(
                    tiles_s[0][:, t, :],
                    x_t[s][t][:],
                    AF.Identity,
                    bias=B_all[:, j : j + 1],
                    scale=A_all[:, j : j + 1],
                )
            else:
                nc.vector.tensor_scalar(
                    out=tiles_s[0][:, t, :],
                    in0=x_t[s][t][:],
                    scalar1=A_all[:, j : j + 1],
                    scalar2=B_all[:, j : j + 1],
                    op0=ALU.mult,
                    op1=ALU.add,
                )

    def prep0_units(out_tiles):
        """Sample 0: emit only pair-0's q/k (o=0/4) inline so attention can
        start immediately, then yield the rest v-first (attn(0,0)'s attn@v
        drain needs vt before later prep matmuls appear in the in-order PE
        stream)."""
        t0 = alloc_prep_tiles(0, out_tiles)
        emit_h(0, t0, use_act=True)
        for o in (0, CT):
            emit_qk_unit(t0, o)
        for mp in range(0, MT, 2):
            emit_v_unit(t0, mp)
            yield 1
        for o in (1, 1 + CT, 2, 2 + CT, 3, 3 + CT):
            emit_qk_unit(t0, o)
            yield 1

    def prep_units(s, out_tiles):
        """Generator: stats + groupnorm-apply + qkv/v matmuls for sample s,
        yielding between units so the caller can interleave."""
        emit_stats(s)
        emit_ab(s)
        yield 1
        t1 = alloc_prep_tiles(s, out_tiles)
        emit_h(s, t1)
        yield 1
        for o in range(2 * CT):
            emit_qk_unit(t1, o)
            yield 1
        for mp in range(0, MT, 2):
            emit_v_unit(t1, mp)
            yield 1

    def emit_norm2(p, ao_sb, avs, nh):
        """Normalize both heads' [65,512] attn@v accumulators for one n-half:
        batched reciprocal, per-head broadcast (split keeps latency low)."""
        nsl = slice(nh * 512, (nh + 1) * 512)
        cs2 = rcppool.tile([1, 2, 512], F32, tag="cs", name="cs")
        for hh in range(2):
            # custom-DVE recip misreads PSUM sources on HW: SBUF-bounce
            nc.vector.tensor_copy(cs2[:, hh, :], avs[hh][D : D + 1, :])
        rcp2 = rcppool.tile([1, 2, 512], F32, tag="rcp", name="rcp")
        nc.vector.reciprocal_approx_fast(rcp2[:], cs2[:])
        for hh in range(2):
            rb = rbpool.tile([64, 512], F32, tag="rb", name="rb")
            nc.gpsimd.partition_broadcast(rb[:], rcp2[:, hh, :])
            nc.vector.tensor_mul(
                ao_sb[hh * 64 : (hh + 1) * 64, p, nsl], avs[hh][0:D, :], rb[:]
            )

    def emit_av_mms(pending, nh):
        s, p, at, vt_sb, ao_sb = pending
        nsl = slice(nh * 512, (nh + 1) * 512)
        avs = []
        for hh in range(2):
            av = ps_av.tile([D + 1, 512], F32, tag="av", name="av")
            for i in range(4):
                mp = 2 * i
                nc.tensor.matmul(
                    av[:],
                    vt_sb[:, mp : mp + 2, 2 * p + hh, 0 : D + 1],
                    at[:, hh, mp : mp + 2, nsl],
                    start=(i == 0),
                    stop=(i == 3),
                    perf_mode=DR,
                )
            avs.append(av)
        return avs

    def emit_av_hh(pending, nh, hh, st):
        """One head's attn@v for one n-half (4 DoubleRow matmuls) plus its
        denominator bounce; the hh=1 call finishes the batched normalize.
        Split per-head so the PE work spreads across m-steps and never
        starves the exp stream."""
        s, p, at, vt_sb, ao_sb = pending
        nsl = slice(nh * 512, (nh + 1) * 512)
        if hh == 0:
            st["cs2"] = rcppool.tile([1, 2, 512], F32, tag="cs", name="cs")
            st["avs"] = []
        av = ps_av.tile([D + 1, 512], F32, tag="av", name="av")
        for i in range(4):
            mp = 2 * i
            nc.tensor.matmul(
                av[:],
                vt_sb[:, mp : mp + 2, 2 * p + hh, 0 : D + 1],
                at[:, hh, mp : mp + 2, nsl],
                start=(i == 0),
                stop=(i == 3),
                perf_mode=DR,
            )
        nc.vector.tensor_copy(st["cs2"][:, hh, :], av[D : D + 1, :])
        st["avs"].append(av)
        if hh == 1:
            rcp2 = rcppool.tile([1, 2, 512], F32, tag="rcp", name="rcp")
            nc.vector.reciprocal_approx_fast(rcp2[:], st["cs2"][:])
            for j in range(2):
                rb = rbpool.tile([64, 512], F32, tag="rb", name="rb")
                nc.gpsimd.partition_broadcast(rb[:], rcp2[:, j, :])
                nc.vector.tensor_mul(
                    ao_sb[j * 64 : (j + 1) * 64, p, nsl], st["avs"][j][0:D, :], rb[:]
                )

    def emit_av_nh(pending, nh):
        """attn@v for both heads of the pending pair at one n-half."""
        avs = emit_av_mms(pending, nh)
        emit_norm2(pending[1], pending[4], avs, nh)

    def proj_units(s, ao_sb):
        """proj + bias + residual + store for sample s (4 per-t units, each
        covering both n-halves; needs the sample's full ao)."""
        for t in range(CT):
            ps = ps_sc.tile([128, N], F32, tag="sc", name="pjps")
            for nh in range(2):
                nsl = slice(nh * 512, (nh + 1) * 512)
                for kp in range(2):
                    nc.tensor.matmul(
                        ps[:, nsl],
                        wp_sb[:, 2 * kp : 2 * kp + 2, t * 128 : (t + 1) * 128],
                        ao_sb[:, 2 * kp : 2 * kp + 2, nsl],
                        start=(kp == 0),
                        stop=(kp == 1),
                        perf_mode=DR,
                    )
            ot = outwpool.tile([128, N], F32, tag="outw", name="otw")
            nc.vector.scalar_tensor_tensor(
                out=ot[:],
                in0=ps[:],
                scalar=beff_sb[:, t, :],
                in1=x_t[s][t][:],
                op0=ALU.add,
                op1=ALU.add,
            )
            nc.sync.dma_start(out=out_e[s, t * 128 : (t + 1) * 128, :], in_=ot[:])
            yield 1

    def attn_pair(
        s,
        p,
        tiles,
        pending,
        ao_sb,
        prep_gen,
        proj_gen,
        proj_slots=(),
    ):
        """Scores + exp for pair p of sample s; interleaves the pending
        pair's attn@v, next-sample prep, and prev-sample proj units.
        early_avs: 4 open PSUM accumulators for THIS pair's attn@v, drained
        per m-pair right behind the exps (used for the final pair, where no
        successor m-loop exists to hide the drain)."""
        q_sb, k_sb, vt_sb = tiles
        at = atpool.tile([128, 2, MT, N], FP8E5, tag="at", name=f"at{s}{p}")

        def emit_sc(m, hh):
            base = hh * 64
            sc = ps_sc.tile([128, N], F32, tag="sc", name="sc")
            for nh in range(2):
                nsl = slice(nh * 512, (nh + 1) * 512)
                nc.tensor.matmul(
                    sc[:, nsl],
                    k_sb[base : base + 64, p, m * 128 : (m + 1) * 128],
                    q_sb[base : base + 64, p, nsl],
                    start=True,
                    stop=True,
                    tile_position=(base, 0),
                )
            return sc

        for m in range(MT):
            for hh in range(2):
                sc = emit_sc(m, hh)
                nc.scalar.activation(at[:, hh, m, :], sc[:], AF.Exp)
            if pending is not None and m in (2, 5):
                emit_av_nh(pending, 0 if m == 2 else 1)
            if prep_gen is not None:
                next(prep_gen, None)
            if proj_gen is not None and m in proj_slots:
                next(proj_gen, None)
        return (s, p, at, vt_sb, ao_sb)

    # ---- drive ----
    import itertools

    tiles = [None, None]
    emit_stats(0)
    emit_ab(0)
    prep_chain = itertools.chain(prep0_units(tiles), prep_units(1, tiles))
    next(prep_chain)  # prime: emits sample-0 tiles, h, pair-0 q/k, v m=0
    proj0 = None
    pending = None
    ao = [None, None]
    for s in range(BS):
        ao[s] = aopool.tile([128, CT, N], FP8, tag="ao", name=f"ao{s}")
        for p in range(HEADS // 2):
            prep_gen = prep_chain if s == 0 else None
            proj_gen = proj0 if s == 1 else None
            # per-t proj units need the previous sample's FULL ao: pair
            # (0,3)'s nh=1 attn@v normalizes at (1,0) m=5, so slots start
            # at m=6 of (1,0).
            slots = (6, 7) if p == 0 else (1, 4)
            pending = attn_pair(
                s, p, tiles[s], pending, ao[s], prep_gen, proj_gen, slots
            )
        if s == 0:
            for _ in prep_chain:
                pass
            prep_chain = None
            proj0 = proj_units(0, ao[0])
    for _ in proj0:
        pass
    # tail: the final pair's nh=1 attn@v matmuls run while the
    # pre-accumulated nh=0 half normalizes; proj(1) follows n-half by
    # n-half (proj units only read their own n columns of ao).
    avs0 = emit_av_mms(pending, 0)
    avs1 = []
    for hh in range(2):
        u = ps_sc.tile([128, N], F32, tag="sc", name=f"avu{hh}")
        for i in range(4):
            mp = 2 * i
            nc.tensor.matmul(
                u[0 : D + 1, 0:512],
                tiles[1][2][:, mp : mp + 2, 6 + hh, 0 : D + 1],
                pending[2][:, hh, mp : mp + 2, 512:1024],
                start=(i == 0),
                stop=(i == 3),
                perf_mode=DR,
            )
        avs1.append(u[0 : D + 1, 0:512])
    emit_norm2(3, ao[1], avs0, 0)
    emit_norm2(3, ao[1], avs1, 1)
    for _ in proj_units(1, ao[1]):
        pass


def build_bass(for_sim: bool = False) -> bass.Bass:
    if for_sim:
        nc = bacc.Bacc(None, target_bir_lowering=False, debug=True)
    else:
        nc = bacc.Bacc()
    te = {
        "x": nc.declare_dram_parameter("x", [BS, C, N], F32, isOutput=False),
        "wqkT": nc.declare_dram_parameter("wqkT", [128, CT, 2 * C], FP8, isOutput=False),
        "wvT": nc.declare_dram_parameter("wvT", [128, CT, C], FP8, isOutput=False),
        "wpT": nc.declare_dram_parameter("wpT", [128, CT, C], FP8, isOutput=False),
        "bq": nc.declare_dram_parameter("bq", [128, CT, 1], F32, isOutput=False),
        "beff": nc.declare_dram_parameter("beff", [128, CT, 1], F32, isOutput=False),
        "gamma": nc.declare_dram_parameter("gamma", [128, CT], F32, isOutput=False),
        "beta": nc.declare_dram_parameter("beta", [128, CT], F32, isOutput=False),
        "selT": nc.declare_dram_parameter("selT", [128, 128], F32R, isOutput=False),
        "out": nc.declare_dram_parameter("out", [BS, C, N], F32, isOutput=True),
    }
    with tile.TileContext(nc) as tc:
        with ExitStack() as ctx:
            _build_tile(ctx, tc, te)
    if for_sim:
        nc.compile()
    else:
        nc.finalize()
    return nc


def _make_selT() -> np.ndarray:
    sel = np.zeros((128, 128), np.float32)
    for p in range(128):
        g = p // 16
        sel[p, g * 16 : (g + 1) * 16] = 1.0 / 16.0
    return sel


def make_in_maps(inputs: dict) -> list[dict]:
    x = np.ascontiguousarray(np.asarray(inputs["x"], np.float32)).reshape(B, C, N)
    w_qkv = np.asarray(inputs["w_qkv"], np.float32)
    b_qkv = np.asarray(inputs["b_qkv"], np.float32)
    w_proj = np.asarray(inputs["w_proj"], np.float32)
    b_proj = np.asarray(inputs["b_proj"], np.float32)
    gamma = np.asarray(inputs["gamma"], np.float32)
    beta = np.asarray(inputs["beta"], np.float32)

    f8 = ml_dtypes.float8_e4m3

    def ptile(w, width):  # [C, width] -> contiguous [128, CT, width]
        return np.ascontiguousarray(
            w.reshape(CT, 128, width).transpose(1, 0, 2)
        )

    common = {
        "wqkT": ptile(np.ascontiguousarray(w_qkv[: 2 * C, :].T).astype(f8), 2 * C),
        "wvT": ptile(np.ascontiguousarray(w_qkv[2 * C :, :].T).astype(f8), C),
        "wpT": ptile(np.ascontiguousarray(w_proj.T).astype(f8), C),
        "bq": ptile(b_qkv[:C].reshape(C, 1).astype(np.float32), 1),
        "beff": ptile(
            (b_proj + w_proj @ b_qkv[2 * C :]).reshape(C, 1).astype(np.float32), 1
        ),
        "gamma": ptile(gamma.reshape(C, 1).astype(np.float32), 1).reshape(128, CT),
        "beta": ptile(beta.reshape(C, 1).astype(np.float32), 1).reshape(128, CT),
        "selT": _make_selT(),
    }
    return [
        {"x": np.ascontiguousarray(x[i * BS : (i + 1) * BS]), **common}
        for i in range(NCORES)
    ]


def kernel(**inputs) -> np.ndarray:
    global LAST_EXEC_NS, LAST_RESULTS
    nc = build_bass()
    in_maps = make_in_maps(inputs)
    res = run_bass_kernel_spmd(nc, in_maps, list(range(NCORES)))
    LAST_RESULTS = res
    LAST_EXEC_NS = res.exec_time_ns
    out = np.concatenate([np.asarray(res.results[i]["out"]) for i in range(NCORES)], 0)
    return out.reshape(B, C, H, W).astype(np.float32)
